# revision 8
# baseline (speedup 1.0000x reference)
"""Binary-weight 3x3 conv (stride 1, pad 1) on 8 TRN2 NeuronCores.

Strategy: data-parallel over batch (4 images per core), weights replicated.
Per image the conv is 9 shifted [Cin,Cout] matmuls accumulated in PSUM
with channels on the partition dim (NCHW layout already has x[n] as a
[C, H*W] channel-major matrix). The input lives in SBUF as bf16 rows of
width 57: data cols 0..55 plus one zero column that doubles as the next
row's LEFT pad, with zero rows above/below (flat [1 + 58*57 + 1] layout).
Every matmul rhs is then a fully CONTIGUOUS 1D window of N=456 covering
8 output rows (one junk psum column per row, discarded by the PSUM->SBUF
copy).

bf16 vs the old fp32r version: fp32r LDWEIGHTS gets no fast-weight-load
(~2x slower than the 456-col matmul stream) and gates the matmul issue
period; bf16 FWL halves the weight load so the period drops to the pure
stream time (~456 cycles -- at full clock the LDW+swap fully hides under
the previous matmul; measured period == stream time, so no further gain
from weight reuse). bf16 inputs cost ~1.7e-3 rel err (budget 2e-2);
binarized weights (+-1, 0) are exact in bf16. fp8 is a dead end: e4m3
input quantization alone measures 2.7e-2 output error, over budget.
The pad/cast runs on the host: x is cast+padded into the exact SBUF
layout in numpy, so DMA deposits ready-to-matmul tiles and the vector
engine only drains PSUM. Output DMAs are batched per image-couttile
group (except the last group, which stays per-block to keep the tail
short) to cut cross-engine semaphore edges -- the end-of-kernel event-
semaphore teardown scales with them.
"""

import numpy as np
import ml_dtypes

N_CORES = 8
B_PER_CORE = 4  # 32 images / 8 cores
CIN = 256
COUT = 256
H = W = 56
WR = 57  # row pitch: 56 data + 1 shared pad col
XLEN = 1 + 58 * WR + 1  # leading pad slot + 58 rows + trailing slot
RB = 8  # output rows per matmul
NBLK = H // RB  # 7
NFREE = RB * WR  # 456 (8 rows x 57, one junk col per row)

_CACHED = {}


def _build_nc():
    import concourse.mybir as mybir
    from concourse import bacc
    from concourse.tile import TileContext

    f32 = mybir.dt.float32
    bf16 = mybir.dt.bfloat16

    nc = bacc.Bacc("TRN2", target_bir_lowering=False, debug=False)
    xs = nc.dram_tensor(
        "xs", [B_PER_CORE, 2, 128, XLEN], bf16, kind="ExternalInput"
    ).ap()
    wt = nc.dram_tensor("wt", [4, 128, 9, 128], bf16, kind="ExternalInput").ap()
    out = nc.dram_tensor(
        "out", [B_PER_CORE, COUT, H, W], f32, kind="ExternalOutput"
    ).ap()

    with TileContext(nc) as tc:
        with (
            tc.tile_pool(name="wp", bufs=1) as wp,
            tc.tile_pool(name="xp", bufs=8) as xp,
            tc.tile_pool(name="yp", bufs=3) as yp,
            tc.tile_pool(name="yl", bufs=2) as yl,
            tc.tile_pool(name="pp", bufs=8, space="PSUM") as pp,
        ):
            w_sb = wp.tile([128, 4, 9, 128], bf16, name="w_sb")

            xt = {}

            def load_image(n):
                for cit in range(2):
                    t = xp.tile([128, XLEN], bf16, name=f"x_{n}_{cit}", tag="x")
                    xt[(n, cit)] = t
                    src = xs[n, cit]
                    if n == 0 and cit == 0:
                        # tap 0 leads on scalar (tiny) so the first ldweights
                        # unblocks immediately; rest of the cit-0 block follows
                        nc.scalar.dma_start(out=w_sb[:, 0, 0], in_=wt[0, :, 0])
                    half = XLEN // 2
                    if n == 0:
                        # first image lands in quarter chunks on both queues
                        q = XLEN // 4
                        nc.sync.dma_start(out=t[:, :q], in_=src[:, :q])
                        nc.scalar.dma_start(out=t[:, q : 2 * q], in_=src[:, q : 2 * q])
                        nc.sync.dma_start(out=t[:, 2 * q : 3 * q], in_=src[:, 2 * q : 3 * q])
                        nc.scalar.dma_start(out=t[:, 3 * q :], in_=src[:, 3 * q :])
                    else:
                        nc.sync.dma_start(out=t[:, :half], in_=src[:, :half])
                        nc.scalar.dma_start(out=t[:, half:], in_=src[:, half:])
                    if n == 0 and cit == 0:
                        nc.scalar.dma_start(
                            out=w_sb[:, 0, 1:], in_=wt[0, :, 1:]
                        )
                        nc.sync.dma_start(out=w_sb[:, 1], in_=wt[1])
                    if n == 0 and cit == 1:
                        nc.sync.dma_start(out=w_sb[:, 2], in_=wt[2])
                        nc.scalar.dma_start(out=w_sb[:, 3], in_=wt[3])

            for _n in range(B_PER_CORE):
                load_image(_n)

            def rhs_ap(n, cit, h0, kh, kw):
                o = (h0 + kh) * WR + kw
                return xt[(n, cit)][:, o : o + NFREE]

            def emit_mms(n, ct, blk, ps):
                """18 accumulating matmuls into one PSUM bank."""
                h0 = blk * RB
                for idx, (cit, k) in enumerate(
                    [(cit, k) for cit in range(2) for k in range(9)]
                ):
                    kh, kw = divmod(k, 3)
                    nc.tensor.matmul(
                        ps[:],
                        lhsT=w_sb[:, ct * 2 + cit, k, :],
                        rhs=rhs_ap(n, cit, h0, kh, kw),
                        start=(idx == 0),
                        stop=(idx == 17),
                    )

            group_i = 0
            n_groups = B_PER_CORE * 2
            for n in range(B_PER_CORE):
                for ct in range(2):
                    if group_i == 0:
                        # cin-tile-major over the whole group (long runway on
                        # cin tile 0 while cin tile 1 finishes DMA); banks 4-6
                        # read the tail quarters of image 0, so they start 3
                        # taps late to chase the DMA wavefront instead of
                        # stalling on its completion semaphores
                        LAG = 3
                        taps = [(c, k) for c in range(2) for k in range(9)]
                        pss = [
                            pp.tile([128, NFREE], f32, name=f"ps{blk}", tag="ps")
                            for blk in range(NBLK)
                        ]
                        for s in range(18 + LAG):
                            for blk in range(NBLK):
                                idx = s if blk < 4 else s - LAG
                                if not (0 <= idx < 18):
                                    continue
                                cit, k = taps[idx]
                                kh, kw = divmod(k, 3)
                                nc.tensor.matmul(
                                    pss[blk][:],
                                    lhsT=w_sb[:, ct * 2 + cit, k, :],
                                    rhs=rhs_ap(n, cit, blk * RB, kh, kw),
                                    start=(idx == 0),
                                    stop=(idx == 17),
                                )
                        yg = yp.tile([128, H * W], f32, name="yg", tag="yg")
                        for blk in range(NBLK):
                            valid = pss[blk].rearrange(
                                "p (h w) -> p h w", w=WR
                            )[:, :, :W]
                            nc.vector.tensor_copy(
                                out=yg[:, blk * RB * W : (blk + 1) * RB * W],
                                in_=valid,
                            )
                        nc.sync.dma_start(
                            out=out[n, ct * 128 : (ct + 1) * 128], in_=yg[:]
                        )
                    elif group_i < n_groups - 1:
                        # one batched output DMA per group, alternating queues
                        yg = yp.tile([128, H * W], f32, name="yg", tag="yg")
                        for blk in range(NBLK):
                            ps = pp.tile([128, NFREE], f32, name="ps", tag="ps")
                            emit_mms(n, ct, blk, ps)
                            valid = ps.rearrange("p (h w) -> p h w", w=WR)[
                                :, :, :W
                            ]
                            nc.vector.tensor_copy(
                                out=yg[:, blk * RB * W : (blk + 1) * RB * W],
                                in_=valid,
                            )
                        eng = nc.sync if group_i % 2 == 0 else nc.scalar
                        eng.dma_start(
                            out=out[n, ct * 128 : (ct + 1) * 128], in_=yg[:]
                        )
                    else:
                        # last group: per-block drains + DMAs keep the tail
                        # short (no 4.5us batched DMA after the last matmul)
                        for blk in range(NBLK):
                            ps = pp.tile([128, NFREE], f32, name="ps", tag="ps")
                            emit_mms(n, ct, blk, ps)
                            y = yl.tile([128, RB * W], f32, name="y", tag="y")
                            valid = ps.rearrange("p (h w) -> p h w", w=WR)[
                                :, :, :W
                            ]
                            nc.vector.tensor_copy(out=y[:], in_=valid)
                            h0 = blk * RB
                            eng = nc.sync if blk % 2 == 0 else nc.scalar
                            eng.dma_start(
                                out=out[
                                    n, ct * 128 : (ct + 1) * 128, h0 : h0 + RB, :
                                ],
                                in_=y[:],
                            )
                    group_i += 1
    nc.compile()
    return nc


def _get_nc():
    if "nc" not in _CACHED:
        _CACHED["nc"] = _build_nc()
    return _CACHED["nc"]


def _prep_weights(W_arr):
    Wb = np.sign(np.asarray(W_arr, dtype=np.float32))
    # [co, ci, kh, kw] -> [ct, cit, ci, k, co] -> [4, 128, 9, 128]
    wt = (
        Wb.reshape(2, 128, 2, 128, 9)
        .transpose(0, 2, 3, 4, 1)
        .reshape(4, 128, 9, 128)
    )
    return np.ascontiguousarray(wt).astype(ml_dtypes.bfloat16)


def _prep_x(x):
    """Cast to bf16 and lay out into the padded pitch-57 SBUF image."""
    x = np.asarray(x, dtype=np.float32).astype(ml_dtypes.bfloat16)
    B = x.shape[0]
    xp = np.zeros((B, 2, 128, XLEN), dtype=ml_dtypes.bfloat16)
    # data row r (0..55) lives at flat offset 1 + 57*(r+1) .. +55; col 56 of
    # each 57-wide row plus flat[0:58] and the tail stay zero (the halo).
    view = xp[:, :, :, 1 + WR : 1 + 57 * WR].reshape(B, 2, 128, 56, WR)
    view[..., :W] = x.reshape(B, 2, 128, H, W)
    return xp


def run(x, W, trace=False, trace_kwargs=None):
    from concourse.bass_utils import run_bass_kernel_spmd

    xp = _prep_x(x)
    wt = _prep_weights(W)
    nc = _get_nc()
    in_maps = [
        {
            "xs": np.ascontiguousarray(xp[i * B_PER_CORE : (i + 1) * B_PER_CORE]),
            "wt": wt,
        }
        for i in range(N_CORES)
    ]
    res = run_bass_kernel_spmd(
        nc,
        in_maps,
        list(range(N_CORES)),
        trace=trace,
        trace_kwargs=trace_kwargs or {},
    )
    out = np.concatenate([np.asarray(res.results[i]["out"]) for i in range(N_CORES)])
    return out, res


def kernel(x, W):
    out, _ = run(x, W, trace=False)
    return out


# revision 9
# speedup vs baseline: 1.6859x; 1.6859x over previous
"""Binary-weight 3x3 conv (stride 1, pad 1) on 8 TRN2 NeuronCores.

Strategy: data-parallel over batch (4 images per core), weights replicated.
Per image the conv is 9 shifted [Cin,Cout] matmuls accumulated in PSUM
with channels on the partition dim (NCHW layout already has x[n] as a
[C, H*W] channel-major matrix). The input lives in SBUF as bf16 rows of
width 57: data cols 0..55 plus one zero column that doubles as the next
row's LEFT pad, with zero rows above/below (flat [1 + 58*57 + 1] layout).
Every matmul rhs is then a fully CONTIGUOUS 1D window of N=456 covering
8 output rows (one junk psum column per row, discarded by the PSUM->SBUF
copy).

bf16 vs the old fp32r version: fp32r LDWEIGHTS gets no fast-weight-load
(~2x slower than the 456-col matmul stream) and gates the matmul issue
period; bf16 FWL halves the weight load so the period drops to the pure
stream time (~456 cycles -- at full clock the LDW+swap fully hides under
the previous matmul; measured period == stream time, so no further gain
from weight reuse). bf16 inputs cost ~1.7e-3 rel err (budget 2e-2);
binarized weights (+-1, 0) are exact in bf16. fp8 is a dead end: e4m3
input quantization alone measures 2.7e-2 output error, over budget.
The pad/cast runs on the host: x is cast+padded into the exact SBUF
layout in numpy, so DMA deposits ready-to-matmul tiles and the vector
engine only drains PSUM. Output DMAs are batched per image-couttile
group (except the last group, which stays per-block to keep the tail
short) to cut cross-engine semaphore edges -- the end-of-kernel event-
semaphore teardown scales with them.
"""

import numpy as np
import ml_dtypes

N_CORES = 8
B_PER_CORE = 4  # 32 images / 8 cores
CIN = 256
COUT = 256
H = W = 56
WR = 57  # row pitch: 56 data + 1 shared pad col
XLEN = 1 + 58 * WR + 1  # leading pad slot + 58 rows + trailing slot
RB = 8  # output rows per matmul
NBLK = H // RB  # 7
NFREE = RB * WR  # 456 (8 rows x 57, one junk col per row)

_CACHED = {}


def _build_nc():
    import concourse.mybir as mybir
    from concourse import bacc
    from concourse.tile import TileContext

    f32 = mybir.dt.float32
    bf16 = mybir.dt.bfloat16

    nc = bacc.Bacc("TRN2", target_bir_lowering=False, debug=False)
    xs = nc.dram_tensor(
        "xs", [B_PER_CORE, 2, 128, XLEN], bf16, kind="ExternalInput"
    ).ap()
    wt = nc.dram_tensor("wt", [4, 128, 9, 128], bf16, kind="ExternalInput").ap()
    out = nc.dram_tensor(
        "out", [B_PER_CORE, COUT, H, W], f32, kind="ExternalOutput"
    ).ap()

    with TileContext(nc) as tc:
        with (
            tc.tile_pool(name="wp", bufs=1) as wp,
            tc.tile_pool(name="xp", bufs=8) as xp,
            tc.tile_pool(name="yp", bufs=3) as yp,
            tc.tile_pool(name="yl", bufs=2) as yl,
            tc.tile_pool(name="pp", bufs=8, space="PSUM") as pp,
        ):
            w_sb = wp.tile([128, 4, 9, 128], bf16, name="w_sb")

            xt = {}

            def load_image(n):
                for cit in range(2):
                    t = xp.tile([128, XLEN], bf16, name=f"x_{n}_{cit}", tag="x")
                    xt[(n, cit)] = t
                    src = xs[n, cit]
                    if n == 0 and cit == 0:
                        # tap 0 leads on scalar (tiny) so the first ldweights
                        # unblocks immediately; rest of the cit-0 block follows
                        nc.scalar.dma_start(out=w_sb[:, 0, 0], in_=wt[0, :, 0])
                    half = XLEN // 2
                    if n == 0:
                        # first image lands in quarter chunks on both queues
                        q = XLEN // 4
                        nc.sync.dma_start(out=t[:, :q], in_=src[:, :q])
                        nc.scalar.dma_start(out=t[:, q : 2 * q], in_=src[:, q : 2 * q])
                        nc.sync.dma_start(out=t[:, 2 * q : 3 * q], in_=src[:, 2 * q : 3 * q])
                        nc.scalar.dma_start(out=t[:, 3 * q :], in_=src[:, 3 * q :])
                    else:
                        nc.sync.dma_start(out=t[:, :half], in_=src[:, :half])
                        nc.scalar.dma_start(out=t[:, half:], in_=src[:, half:])
                    if n == 0 and cit == 0:
                        nc.scalar.dma_start(
                            out=w_sb[:, 0, 1:], in_=wt[0, :, 1:]
                        )
                        nc.sync.dma_start(out=w_sb[:, 1], in_=wt[1])
                    if n == 0 and cit == 1:
                        nc.sync.dma_start(out=w_sb[:, 2], in_=wt[2])
                        nc.scalar.dma_start(out=w_sb[:, 3], in_=wt[3])

            for _n in range(B_PER_CORE):
                load_image(_n)

            def rhs_ap(n, cit, h0, kh, kw):
                o = (h0 + kh) * WR + kw
                return xt[(n, cit)][:, o : o + NFREE]

            def emit_mms(n, ct, blk, ps):
                """18 accumulating matmuls into one PSUM bank."""
                h0 = blk * RB
                for idx, (cit, k) in enumerate(
                    [(cit, k) for cit in range(2) for k in range(9)]
                ):
                    kh, kw = divmod(k, 3)
                    nc.tensor.matmul(
                        ps[:],
                        lhsT=w_sb[:, ct * 2 + cit, k, :],
                        rhs=rhs_ap(n, cit, h0, kh, kw),
                        start=(idx == 0),
                        stop=(idx == 17),
                    )

            group_i = 0
            n_groups = B_PER_CORE * 2
            for n in range(B_PER_CORE):
                for ct in range(2):
                    if group_i == 0:
                        # cin-tile-major over the whole group: 63 matmuls of
                        # runway on cin tile 0 while cin tile 1 finishes DMA
                        pss = [
                            pp.tile([128, NFREE], f32, name=f"ps{blk}", tag="ps")
                            for blk in range(NBLK)
                        ]
                        for idx, (cit, k) in enumerate(
                            [(c, k) for c in range(2) for k in range(9)]
                        ):
                            kh, kw = divmod(k, 3)
                            for blk in range(NBLK):
                                nc.tensor.matmul(
                                    pss[blk][:],
                                    lhsT=w_sb[:, ct * 2 + cit, k, :],
                                    rhs=rhs_ap(n, cit, blk * RB, kh, kw),
                                    start=(idx == 0),
                                    stop=(idx == 17),
                                )
                        yg = yp.tile([128, H * W], f32, name="yg", tag="yg")
                        for blk in range(NBLK):
                            valid = pss[blk].rearrange(
                                "p (h w) -> p h w", w=WR
                            )[:, :, :W]
                            nc.vector.tensor_copy(
                                out=yg[:, blk * RB * W : (blk + 1) * RB * W],
                                in_=valid,
                            )
                        nc.sync.dma_start(
                            out=out[n, ct * 128 : (ct + 1) * 128], in_=yg[:]
                        )
                    elif group_i < n_groups - 1:
                        # one batched output DMA per group, alternating queues
                        yg = yp.tile([128, H * W], f32, name="yg", tag="yg")
                        for blk in range(NBLK):
                            ps = pp.tile([128, NFREE], f32, name="ps", tag="ps")
                            emit_mms(n, ct, blk, ps)
                            valid = ps.rearrange("p (h w) -> p h w", w=WR)[
                                :, :, :W
                            ]
                            nc.vector.tensor_copy(
                                out=yg[:, blk * RB * W : (blk + 1) * RB * W],
                                in_=valid,
                            )
                        eng = nc.sync if group_i % 2 == 0 else nc.scalar
                        eng.dma_start(
                            out=out[n, ct * 128 : (ct + 1) * 128], in_=yg[:]
                        )
                    else:
                        # last group: per-block drains + DMAs keep the tail
                        # short (no 4.5us batched DMA after the last matmul)
                        for blk in range(NBLK):
                            ps = pp.tile([128, NFREE], f32, name="ps", tag="ps")
                            emit_mms(n, ct, blk, ps)
                            y = yl.tile([128, RB * W], f32, name="y", tag="y")
                            valid = ps.rearrange("p (h w) -> p h w", w=WR)[
                                :, :, :W
                            ]
                            nc.vector.tensor_copy(out=y[:], in_=valid)
                            h0 = blk * RB
                            eng = nc.sync if blk % 2 == 0 else nc.scalar
                            eng.dma_start(
                                out=out[
                                    n, ct * 128 : (ct + 1) * 128, h0 : h0 + RB, :
                                ],
                                in_=y[:],
                            )
                    group_i += 1
    nc.compile()
    return nc


def _get_nc():
    if "nc" not in _CACHED:
        _CACHED["nc"] = _build_nc()
    return _CACHED["nc"]


def _prep_weights(W_arr):
    Wb = np.sign(np.asarray(W_arr, dtype=np.float32))
    # [co, ci, kh, kw] -> [ct, cit, ci, k, co] -> [4, 128, 9, 128]
    wt = (
        Wb.reshape(2, 128, 2, 128, 9)
        .transpose(0, 2, 3, 4, 1)
        .reshape(4, 128, 9, 128)
    )
    return np.ascontiguousarray(wt).astype(ml_dtypes.bfloat16)


def _prep_x(x):
    """Cast to bf16 and lay out into the padded pitch-57 SBUF image."""
    x = np.asarray(x, dtype=np.float32).astype(ml_dtypes.bfloat16)
    B = x.shape[0]
    xp = np.zeros((B, 2, 128, XLEN), dtype=ml_dtypes.bfloat16)
    # data row r (0..55) lives at flat offset 1 + 57*(r+1) .. +55; col 56 of
    # each 57-wide row plus flat[0:58] and the tail stay zero (the halo).
    view = xp[:, :, :, 1 + WR : 1 + 57 * WR].reshape(B, 2, 128, 56, WR)
    view[..., :W] = x.reshape(B, 2, 128, H, W)
    return xp


def run(x, W, trace=False, trace_kwargs=None):
    from concourse.bass_utils import run_bass_kernel_spmd

    xp = _prep_x(x)
    wt = _prep_weights(W)
    nc = _get_nc()
    in_maps = [
        {
            "xs": np.ascontiguousarray(xp[i * B_PER_CORE : (i + 1) * B_PER_CORE]),
            "wt": wt,
        }
        for i in range(N_CORES)
    ]
    res = run_bass_kernel_spmd(
        nc,
        in_maps,
        list(range(N_CORES)),
        trace=trace,
        trace_kwargs=trace_kwargs or {},
    )
    out = np.concatenate([np.asarray(res.results[i]["out"]) for i in range(N_CORES)])
    return out, res


def kernel(x, W):
    out, _ = run(x, W, trace=False)
    return out
